# revision 2
# baseline (speedup 1.0000x reference)
"""Distributed multi-head attention block on 8 TRN2 NeuronCores.

Reference computation (B=2, S=2048, D=1024, H=16, DH=64):
    q = split_heads(q_ @ Wq + bq); k = ...; v = ...
    attn = softmax(q k^T / 8)  (mask is all-ones -> identity row mask)
    out = (merge_heads(attn @ v) + q_) @ Wf + bf

Sharding: 16 heads split 8 ways (2 heads / core); each core handles BOTH
batches.  The "virtual q" axis is b-major: vq = b*2048 + s (4096 total).

Per core c (heads 2c, 2c+1; d-dims 128c..128c+128):
  1. Projections (bf16): QT/KT [128 dh, 4096 vq], V [vk, 128 dh] from the
     transposed inputs xq/xk/xv [1024 din, 4096] and weight slices.
  2. Attention, transposed formulation: ST[k, q] = KT^T Q per (head, k-tile,
     q-chunk); exp via ScalarE straight from PSUM (scale=1/8 folded in);
     OT[dh, q] = V^T P accumulated over k-tiles; row-sums via ones-matmuls;
     softmax denominators broadcast with K=1 matmuls; normalize + residual
     on VectorE -> ZT_local [128 d, 4096 vq] (bf16).
  3. One 8-core AllToAll exchanges q-slices: core c ends with
     ZT_full [1024 d, 512] for virtual q chunk c, computes the final fc
     Y = ZT_full^T @ Wf (f32 out) for its 512 rows.

Host side: casts/transposes inputs (numpy), feeds per-core shards, places
each core's [512, 1024] output chunk, adds bf.  If the mask is not all-ones
(never happens with this problem's generator), falls back to a numpy
reference implementation.
"""

import sys

sys.path.insert(0, "/opt/trn_rl_repo")

import ml_dtypes
import numpy as np

import concourse.bass as bass
import concourse.tile as tile
from concourse import bacc, mybir
from concourse.bass_utils import run_bass_kernel_spmd

B, S, D, H = 2, 2048, 1024, 16
DH = D // H  # 64
N_CORES = 8
VQ = B * S  # 4096 virtual q (b-major)
NQC = VQ // 512  # 8 q-chunks of 512
NKT = S // 128  # 16 k-tiles per batch
NDIN = D // 128  # 8 din tiles

BF16 = mybir.dt.bfloat16
F32 = mybir.dt.float32
AF = mybir.ActivationFunctionType
ALU = mybir.AluOpType
BF16NP = ml_dtypes.bfloat16

_CACHE = {}


def _build():
    nc = bacc.Bacc(None, target_bir_lowering=False)

    xq = nc.declare_dram_parameter("xq", [D, VQ], BF16, isOutput=False)
    xk = nc.declare_dram_parameter("xk", [D, VQ], BF16, isOutput=False)
    xv = nc.declare_dram_parameter("xv", [D, VQ], BF16, isOutput=False)
    xres = nc.declare_dram_parameter("xres", [128, VQ], BF16, isOutput=False)
    wq = nc.declare_dram_parameter("wq", [D, 128], BF16, isOutput=False)
    wk = nc.declare_dram_parameter("wk", [D, 128], BF16, isOutput=False)
    wv = nc.declare_dram_parameter("wv", [D, 128], BF16, isOutput=False)
    wf = nc.declare_dram_parameter("wf", [D, D], BF16, isOutput=False)
    bq = nc.declare_dram_parameter("bq", [128, 1], F32, isOutput=False)
    bk = nc.declare_dram_parameter("bk", [128, 1], F32, isOutput=False)
    bv = nc.declare_dram_parameter("bv", [1, 128], BF16, isOutput=False)
    out = nc.declare_dram_parameter("out", [512, D], F32, isOutput=True)

    with tile.TileContext(nc) as tc:
        with (
            tc.tile_pool(name="persist", bufs=1) as sbp,
            tc.tile_pool(name="dram", bufs=1, space="DRAM") as dram,
        ):
            # ---- persistent SBUF tensors ----
            qt_sb = sbp.tile([128, VQ], BF16)  # [2 heads x 64 dh, vq]
            kt_sb = sbp.tile([128, VQ], BF16)  # [2 heads x 64 dh, vkey]
            v_sb = sbp.tile([128, 32 * 128], BF16)  # [k in tile, (b,kt) x 128 dh]
            wq_sb = sbp.tile([128, NDIN * 128], BF16)
            wk_sb = sbp.tile([128, NDIN * 128], BF16)
            wv_sb = sbp.tile([128, NDIN * 128], BF16)
            wf_sb = sbp.tile([128, NDIN * 1024], BF16)
            xres_sb = sbp.tile([128, VQ], BF16)
            zt_local = sbp.tile([128, VQ], BF16)
            zt_full = sbp.tile([128, NQC * 512], BF16)
            bq_sb = sbp.tile([128, 1], F32)
            bk_sb = sbp.tile([128, 1], F32)
            bv_sb = sbp.tile([1, 128], BF16)
            ones_col = sbp.tile([128, 1], BF16)
            ones_sb = sbp.tile([128, 128], BF16)
            nc.vector.memset(ones_col[:], 1.0)
            nc.vector.memset(ones_sb[:], 1.0)

            for j in range(NDIN):
                nc.gpsimd.dma_start(wq_sb[:, 128 * j : 128 * (j + 1)], wq[128 * j : 128 * (j + 1), :])
                nc.gpsimd.dma_start(wk_sb[:, 128 * j : 128 * (j + 1)], wk[128 * j : 128 * (j + 1), :])
                nc.gpsimd.dma_start(wv_sb[:, 128 * j : 128 * (j + 1)], wv[128 * j : 128 * (j + 1), :])
                nc.gpsimd.dma_start(wf_sb[:, 1024 * j : 1024 * (j + 1)], wf[128 * j : 128 * (j + 1), :])
            nc.gpsimd.dma_start(bq_sb[:], bq[:])
            nc.gpsimd.dma_start(bk_sb[:], bk[:])
            nc.gpsimd.dma_start(bv_sb[:], bv[:])
            nc.gpsimd.dma_start(xres_sb[:], xres[:])

            # =================== phase 1: projections ===================
            with (
                tc.tile_pool(name="xin", bufs=8) as xin,
                tc.tile_pool(name="ps1", bufs=8, space="PSUM") as ps1,
            ):
                # ---- V = xv^T @ wv  -> [vk, 128 dh], natural layout ----
                xv_tiles = []
                for din in range(NDIN):
                    xt = xin.tile([128, VQ], BF16, name=f"xv{din}", tag="x")
                    nc.gpsimd.dma_start(xt[:], xv[128 * din : 128 * (din + 1), :])
                    xv_tiles.append(xt)
                for grp in range(4):  # 8 k-tiles per group
                    vps = []
                    for i in range(8):
                        vkt = grp * 8 + i
                        vp = ps1.tile([128, 512], F32, name=f"vps{vkt}", tag="ps")
                        vps.append(vp)
                        for din in range(NDIN):
                            nc.tensor.matmul(
                                vp[:, 0:128],
                                lhsT=xv_tiles[din][:, 128 * vkt : 128 * (vkt + 1)],
                                rhs=wv_sb[:, 128 * din : 128 * (din + 1)],
                                start=(din == 0),
                                stop=False,
                            )
                        # + bv broadcast over keys (rank-1)
                        nc.tensor.matmul(
                            vp[:, 0:128],
                            lhsT=ones_sb[0:1, :],
                            rhs=bv_sb[:],
                            start=False,
                            stop=True,
                        )
                    for i in range(8):
                        vkt = grp * 8 + i
                        nc.vector.tensor_copy(
                            v_sb[:, 128 * vkt : 128 * (vkt + 1)], vps[i][:, 0:128]
                        )

                # ---- QT = wq^T @ xq -> [128 dh, vq] ----
                for name, xdram, w_sb, b_sb, dst in (
                    ("q", xq, wq_sb, bq_sb, qt_sb),
                    ("k", xk, wk_sb, bk_sb, kt_sb),
                ):
                    pts = [
                        ps1.tile([128, 512], F32, name=f"{name}ps{qc}", tag="ps")
                        for qc in range(NQC)
                    ]
                    for din in range(NDIN):
                        xt = xin.tile([128, VQ], BF16, name=f"x{name}{din}", tag="x")
                        nc.gpsimd.dma_start(xt[:], xdram[128 * din : 128 * (din + 1), :])
                        for qc in range(NQC):
                            nc.tensor.matmul(
                                pts[qc][:],
                                lhsT=w_sb[:, 128 * din : 128 * (din + 1)],
                                rhs=xt[:, 512 * qc : 512 * (qc + 1)],
                                start=(din == 0),
                                stop=(din == NDIN - 1),
                            )
                    for qc in range(NQC):
                        nc.vector.tensor_scalar_add(
                            dst[:, 512 * qc : 512 * (qc + 1)], pts[qc][:], b_sb[:]
                        )

            # =================== phase 2: attention ===================
            with (
                tc.tile_pool(name="stb", bufs=1, space="PSUM") as stb,  # 4 banks
                tc.tile_pool(name="sts", bufs=1, space="PSUM") as sts,  # 2 banks
                tc.tile_pool(name="otp", bufs=1, space="PSUM") as otp,  # 1 bank
                tc.tile_pool(name="rsp", bufs=1, space="PSUM") as rsp,  # 1 bank
                tc.tile_pool(name="ptp", bufs=4) as ptp,
                tc.tile_pool(name="nrm", bufs=2) as nrm,
            ):
                # k-tile schedule per q-chunk: groups of (2,1) k-tiles:
                groups = []
                kt0 = 0
                while kt0 < NKT:
                    if kt0 + 2 < NKT:
                        groups.append((kt0, 2))
                        kt0 += 2
                        groups.append((kt0, 1))
                        kt0 += 1
                    else:
                        groups.append((kt0, NKT - kt0))
                        kt0 += NKT - kt0
                for qc in range(NQC):
                    b = qc // 4
                    q0 = 512 * qc
                    ot = otp.tile([128, 512], F32, name=f"ot{qc}", tag="ot")
                    rs = rsp.tile([128, 512], F32, name=f"rs{qc}", tag="rs")
                    for gi, (k0, gn) in enumerate(groups):
                        pool = stb if gn == 2 else sts
                        st = pool.tile(
                            [128, 1024 * gn], F32, name=f"st{qc}_{gi}",
                            tag="big" if gn == 2 else "small",
                        )
                        pt = ptp.tile(
                            [128, 1024 * gn], BF16, name=f"pt{qc}_{gi}",
                            tag="bigpt" if gn == 2 else "smallpt",
                        )
                        for i in range(gn):
                            kt = k0 + i
                            kk = 2048 * b + 128 * kt
                            for h in range(2):
                                nc.tensor.matmul(
                                    st[:, 1024 * i + 512 * h : 1024 * i + 512 * (h + 1)],
                                    lhsT=kt_sb[64 * h : 64 * (h + 1), kk : kk + 128],
                                    rhs=qt_sb[64 * h : 64 * (h + 1), q0 : q0 + 512],
                                    start=True,
                                    stop=True,
                                )
                        nc.scalar.activation(pt[:], st[:], AF.Exp, scale=0.125)
                        for i in range(gn):
                            kt = k0 + i
                            vk = 128 * (16 * b + kt)
                            first = kt == 0
                            last = kt == NKT - 1
                            for h in range(2):
                                nc.tensor.matmul(
                                    ot[64 * h : 64 * (h + 1), :],
                                    lhsT=v_sb[:, vk + 64 * h : vk + 64 * (h + 1)],
                                    rhs=pt[:, 1024 * i + 512 * h : 1024 * i + 512 * (h + 1)],
                                    start=first,
                                    stop=last,
                                    tile_position=(0, 64 * h),
                                )
                                nc.tensor.matmul(
                                    rs[32 * h : 32 * h + 1, :],
                                    lhsT=ones_col[:],
                                    rhs=pt[:, 1024 * i + 512 * h : 1024 * i + 512 * (h + 1)],
                                    start=first,
                                    stop=last,
                                    tile_position=(0, 32 * h),
                                )
                    # softmax denominators -> broadcast -> normalize + residual
                    rs_bf = nrm.tile([128, 512], BF16, name=f"rsbf{qc}", tag="rsbf")
                    nc.vector.tensor_copy(rs_bf[0:1, :], rs[0:1, :])
                    nc.vector.tensor_copy(rs_bf[32:33, :], rs[32:33, :])
                    bc = rsp.tile([128, 512], F32, name=f"bc{qc}", tag="rs")
                    nc.tensor.matmul(
                        bc[0:64, :], lhsT=ones_sb[0:1, 0:64], rhs=rs_bf[0:1, :],
                        start=True, stop=True, tile_position=(0, 0),
                    )
                    nc.tensor.matmul(
                        bc[64:128, :], lhsT=ones_sb[32:33, 0:64], rhs=rs_bf[32:33, :],
                        start=True, stop=True, tile_position=(32, 64),
                    )
                    recipb = nrm.tile([128, 512], F32, name=f"recipb{qc}", tag="recipb")
                    nc.vector.reciprocal(recipb[:], bc[:])
                    o_tmp = nrm.tile([128, 512], BF16, name=f"otmp{qc}", tag="otmp")
                    nc.vector.tensor_tensor(o_tmp[:], ot[:], recipb[:], ALU.mult)
                    nc.vector.tensor_tensor(
                        zt_local[:, q0 : q0 + 512], o_tmp[:], xres_sb[:, q0 : q0 + 512],
                        ALU.add,
                    )

            # =================== phase 3: A2A + fc ===================
            a2a_in = dram.tile([1024, 512], BF16)
            a2a_out = dram.tile([1024, 512], BF16)
            for j in range(NQC):
                nc.gpsimd.dma_start(
                    a2a_in[128 * j : 128 * (j + 1), :], zt_local[:, 512 * j : 512 * (j + 1)]
                )
            nc.gpsimd.collective_compute(
                "AllToAll",
                ALU.bypass,
                replica_groups=[list(range(N_CORES))],
                ins=[a2a_in.opt()],
                outs=[a2a_out.opt()],
            )
            for j in range(NQC):
                nc.gpsimd.dma_start(
                    zt_full[:, 512 * j : 512 * (j + 1)], a2a_out[128 * j : 128 * (j + 1), :]
                )

            with (
                tc.tile_pool(name="fcps", bufs=4, space="PSUM") as fcps,
                tc.tile_pool(name="ysb", bufs=2) as ysb,
            ):
                for qt in range(4):
                    y = ysb.tile([128, 1024], F32, name=f"y{qt}", tag="y")
                    for nb in range(2):
                        yp = fcps.tile([128, 512], F32, name=f"yp{qt}_{nb}", tag="yp")
                        for j in range(NDIN):
                            nc.tensor.matmul(
                                yp[:],
                                lhsT=zt_full[:, 512 * j + 128 * qt : 512 * j + 128 * (qt + 1)],
                                rhs=wf_sb[:, 1024 * j + 512 * nb : 1024 * j + 512 * (nb + 1)],
                                start=(j == 0),
                                stop=(j == NDIN - 1),
                            )
                        nc.vector.tensor_copy(y[:, 512 * nb : 512 * (nb + 1)], yp[:])
                    nc.gpsimd.dma_start(out[128 * qt : 128 * (qt + 1), :], y[:])

    nc.compile()
    return nc


def _numpy_reference(q_, k_, v_, mask, Wq, bq, Wk, bk, Wv, bv, Wf, bf):
    q_ = np.asarray(q_, np.float32)
    k_ = np.asarray(k_, np.float32)
    v_ = np.asarray(v_, np.float32)
    b = q_.shape[0]

    def split(x):
        return x.reshape(b, -1, H, DH).transpose(0, 2, 1, 3)

    q = split(q_ @ Wq + bq)
    k = split(k_ @ Wk + bk)
    v = split(v_ @ Wv + bv)
    attn = np.einsum("bhqd,bhkd->bhqk", q, k) / np.sqrt(np.float32(DH))
    attn = np.where(np.asarray(mask)[:, None, :, None], attn, np.float32(-1e12))
    attn = attn - attn.max(axis=-1, keepdims=True)
    e = np.exp(attn)
    p = e / e.sum(axis=-1, keepdims=True)
    o = np.einsum("bhqk,bhkd->bhqd", p, v)
    o = o.transpose(0, 2, 1, 3).reshape(b, -1, D)
    return (o + q_) @ Wf + bf


def kernel(q_, k_, v_, mask, Wq, bq, Wk, bk, Wv, bv, Wf, bf):
    mask = np.asarray(mask)
    if not mask.all():
        return _numpy_reference(q_, k_, v_, mask, Wq, bq, Wk, bk, Wv, bv, Wf, bf)

    q_ = np.asarray(q_, np.float32)
    k_ = np.asarray(k_, np.float32)
    v_ = np.asarray(v_, np.float32)

    # transposed, b-major-concatenated inputs (shared across cores)
    xq = np.ascontiguousarray(np.concatenate([q_[b].T for b in range(B)], axis=1)).astype(BF16NP)
    xk = np.ascontiguousarray(np.concatenate([k_[b].T for b in range(B)], axis=1)).astype(BF16NP)
    xv = np.ascontiguousarray(np.concatenate([v_[b].T for b in range(B)], axis=1)).astype(BF16NP)
    wf_b = np.ascontiguousarray(np.asarray(Wf, np.float32)).astype(BF16NP)

    in_maps = []
    for c in range(N_CORES):
        d0 = 128 * c
        in_maps.append(
            {
                "xq": xq,
                "xk": xk,
                "xv": xv,
                "xres": np.ascontiguousarray(xq[d0 : d0 + 128, :]),
                "wq": np.ascontiguousarray(np.asarray(Wq, np.float32)[:, d0 : d0 + 128]).astype(BF16NP),
                "wk": np.ascontiguousarray(np.asarray(Wk, np.float32)[:, d0 : d0 + 128]).astype(BF16NP),
                "wv": np.ascontiguousarray(np.asarray(Wv, np.float32)[:, d0 : d0 + 128]).astype(BF16NP),
                "wf": wf_b,
                "bq": np.ascontiguousarray(np.asarray(bq, np.float32)[d0 : d0 + 128, None]),
                "bk": np.ascontiguousarray(np.asarray(bk, np.float32)[d0 : d0 + 128, None]),
                "bv": np.ascontiguousarray(np.asarray(bv, np.float32)[None, d0 : d0 + 128]).astype(BF16NP),
            }
        )

    if "nc" not in _CACHE:
        _CACHE["nc"] = _build()
    res = run_bass_kernel_spmd(_CACHE["nc"], in_maps, core_ids=list(range(N_CORES)))

    out = np.empty((B, S, D), np.float32)
    for c in range(N_CORES):
        y = res.results[c]["out"]
        out[c // 4, 512 * (c % 4) : 512 * (c % 4 + 1), :] = y
    out += np.asarray(bf, np.float32)[None, None, :]
    return out


if __name__ == "__main__":
    # smoke test with small random data through the numpy fallback shapes
    rng = np.random.default_rng(0)
    args = dict(
        q_=rng.standard_normal((B, S, D), dtype=np.float32),
        k_=rng.standard_normal((B, S, D), dtype=np.float32),
        v_=rng.standard_normal((B, S, D), dtype=np.float32),
        mask=np.ones((B, S), bool),
        Wq=rng.standard_normal((D, D), dtype=np.float32) * 0.02,
        bq=np.zeros(D, np.float32),
        Wk=rng.standard_normal((D, D), dtype=np.float32) * 0.02,
        bk=np.zeros(D, np.float32),
        Wv=rng.standard_normal((D, D), dtype=np.float32) * 0.02,
        bv=np.zeros(D, np.float32),
        Wf=rng.standard_normal((D, D), dtype=np.float32) * 0.02,
        bf=np.zeros(D, np.float32),
    )
    got = kernel(**args)
    want = _numpy_reference(**args)
    rel = np.abs(got - want).max() / np.abs(want).max()
    print("rel_err:", rel)


# revision 4
# speedup vs baseline: 1.2837x; 1.2837x over previous
"""Distributed multi-head attention block on 8 TRN2 NeuronCores.

Reference computation (B=2, S=2048, D=1024, H=16, DH=64):
    q = split_heads(q_ @ Wq + bq); k = ...; v = ...
    attn = softmax(q k^T / 8)  (mask is all-ones -> identity row mask)
    out = (merge_heads(attn @ v) + q_) @ Wf + bf

Sharding: 16 heads split 8 ways (2 heads / core); each core handles BOTH
batches.  The "virtual q" axis is b-major: vq = b*2048 + s (4096 total).

Per core c (heads 2c, 2c+1; d-dims 128c..128c+128):
  1. Projections (bf16): QT/KT [128 dh, 4096 vq], V [vk, 128 dh] from the
     transposed inputs xq/xk/xv [1024 din, 4096] and weight slices.
  2. Attention, transposed formulation: ST[k, q] = KT^T Q per (head, k-tile,
     q-chunk); exp via ScalarE straight from PSUM (scale=1/8 folded in);
     OT[dh, q] = V^T P accumulated over k-tiles; row-sums via ones-matmuls;
     softmax denominators broadcast with K=1 matmuls; normalize + residual
     on VectorE -> ZT_local [128 d, 4096 vq] (bf16).
  3. One 8-core AllToAll exchanges q-slices: core c ends with
     ZT_full [1024 d, 512] for virtual q chunk c, computes the final fc
     Y = ZT_full^T @ Wf (f32 out) for its 512 rows.

Host side: casts/transposes inputs (numpy), feeds per-core shards, places
each core's [512, 1024] output chunk, adds bf.  If the mask is not all-ones
(never happens with this problem's generator), falls back to a numpy
reference implementation.
"""

import sys

sys.path.insert(0, "/opt/trn_rl_repo")

import ml_dtypes
import numpy as np

import concourse.bass as bass
import concourse.tile as tile
from concourse import bacc, mybir
from concourse.bass_utils import run_bass_kernel_spmd

B, S, D, H = 2, 2048, 1024, 16
DH = D // H  # 64
N_CORES = 8
VQ = B * S  # 4096 virtual q (b-major)
NQC = VQ // 512  # 8 q-chunks of 512
NKT = S // 128  # 16 k-tiles per batch
NDIN = D // 128  # 8 din tiles

BF16 = mybir.dt.bfloat16
F32 = mybir.dt.float32
AF = mybir.ActivationFunctionType
ALU = mybir.AluOpType
BF16NP = ml_dtypes.bfloat16

_CACHE = {}


def _build():
    nc = bacc.Bacc(None, target_bir_lowering=False)

    xq = nc.declare_dram_parameter("xq", [D, VQ], BF16, isOutput=False)
    xk = nc.declare_dram_parameter("xk", [D, VQ], BF16, isOutput=False)
    xv = nc.declare_dram_parameter("xv", [D, VQ], BF16, isOutput=False)
    xres = nc.declare_dram_parameter("xres", [128, VQ], BF16, isOutput=False)
    wq = nc.declare_dram_parameter("wq", [D, 128], BF16, isOutput=False)
    wk = nc.declare_dram_parameter("wk", [D, 128], BF16, isOutput=False)
    wv = nc.declare_dram_parameter("wv", [D, 128], BF16, isOutput=False)
    wf = nc.declare_dram_parameter("wf", [D, D], BF16, isOutput=False)
    bq = nc.declare_dram_parameter("bq", [128, 1], F32, isOutput=False)
    bk = nc.declare_dram_parameter("bk", [128, 1], F32, isOutput=False)
    bv = nc.declare_dram_parameter("bv", [1, 128], BF16, isOutput=False)
    out = nc.declare_dram_parameter("out", [512, D], F32, isOutput=True)

    with tile.TileContext(nc) as tc:
        with (
            tc.tile_pool(name="persist", bufs=1) as sbp,
            tc.tile_pool(name="dram", bufs=1, space="DRAM") as dram,
        ):
            # ---- persistent SBUF tensors ----
            qt_sb = sbp.tile([128, VQ], BF16)  # [2 heads x 64 dh, vq]
            kt_sb = sbp.tile([128, VQ], BF16)  # [2 heads x 64 dh, vkey]
            v_sb = sbp.tile([128, 32 * 128], BF16)  # [k in tile, (b,kt) x 128 dh]
            wq_sb = sbp.tile([128, NDIN * 128], BF16)
            wk_sb = sbp.tile([128, NDIN * 128], BF16)
            wv_sb = sbp.tile([128, NDIN * 128], BF16)
            wf_sb = sbp.tile([128, NDIN * 1024], BF16)
            xres_sb = sbp.tile([128, VQ], BF16)
            zt_local = sbp.tile([128, VQ], BF16)
            zt_full = sbp.tile([128, NQC * 512], BF16)
            bq_sb = sbp.tile([128, 1], F32)
            bk_sb = sbp.tile([128, 1], F32)
            bv_sb = sbp.tile([1, 128], BF16)
            ones_col = sbp.tile([128, 1], BF16)
            ones_sb = sbp.tile([128, 128], BF16)
            nc.vector.memset(ones_col[:], 1.0)
            nc.vector.memset(ones_sb[:], 1.0)

            for j in range(NDIN):
                nc.gpsimd.dma_start(wq_sb[:, 128 * j : 128 * (j + 1)], wq[128 * j : 128 * (j + 1), :])
                nc.gpsimd.dma_start(wk_sb[:, 128 * j : 128 * (j + 1)], wk[128 * j : 128 * (j + 1), :])
                nc.gpsimd.dma_start(wv_sb[:, 128 * j : 128 * (j + 1)], wv[128 * j : 128 * (j + 1), :])
                nc.gpsimd.dma_start(wf_sb[:, 1024 * j : 1024 * (j + 1)], wf[128 * j : 128 * (j + 1), :])
            nc.gpsimd.dma_start(bq_sb[:], bq[:])
            nc.gpsimd.dma_start(bk_sb[:], bk[:])
            nc.gpsimd.dma_start(bv_sb[:], bv[:])
            nc.gpsimd.dma_start(xres_sb[:], xres[:])

            # =================== phase 1: projections ===================
            # order: xv first (V pass blocks attention via PSUM banks), then
            # xq -> QT, then xk -> KT (attention starts when KT lands).
            with (
                tc.tile_pool(name="xvp", bufs=8) as xvp,
                tc.tile_pool(name="xin", bufs=4) as xin,
                tc.tile_pool(name="ps1", bufs=8, space="PSUM") as ps1,
            ):
                # ---- V = xv^T @ wv  -> [vk, 128 dh], natural layout ----
                xv_tiles = []
                for din in range(NDIN):
                    xt = xvp.tile([128, VQ], BF16, name=f"xv{din}", tag="xv")
                    nc.gpsimd.dma_start(xt[:], xv[128 * din : 128 * (din + 1), :])
                    xv_tiles.append(xt)
                for grp in range(4):  # 8 k-tiles per group
                    vps = []
                    for i in range(8):
                        vkt = grp * 8 + i
                        vp = ps1.tile([128, 512], F32, name=f"vps{vkt}", tag="ps")
                        vps.append(vp)
                        for din in range(NDIN):
                            nc.tensor.matmul(
                                vp[:, 0:128],
                                lhsT=xv_tiles[din][:, 128 * vkt : 128 * (vkt + 1)],
                                rhs=wv_sb[:, 128 * din : 128 * (din + 1)],
                                start=(din == 0),
                                stop=False,
                            )
                        # + bv broadcast over keys (rank-1)
                        nc.tensor.matmul(
                            vp[:, 0:128],
                            lhsT=ones_sb[0:1, :],
                            rhs=bv_sb[:],
                            start=False,
                            stop=True,
                        )
                    for i in range(8):
                        vkt = grp * 8 + i
                        nc.vector.tensor_copy(
                            v_sb[:, 128 * vkt : 128 * (vkt + 1)], vps[i][:, 0:128]
                        )

                # ---- QT = wq^T @ xq -> [128 dh, vq] ----
                for name, xdram, w_sb, b_sb, dst in (
                    ("q", xq, wq_sb, bq_sb, qt_sb),
                    ("k", xk, wk_sb, bk_sb, kt_sb),
                ):
                    pts = [
                        ps1.tile([128, 512], F32, name=f"{name}ps{qc}", tag="ps")
                        for qc in range(NQC)
                    ]
                    for din in range(NDIN):
                        xt = xin.tile([128, VQ], BF16, name=f"x{name}{din}", tag="x")
                        nc.gpsimd.dma_start(xt[:], xdram[128 * din : 128 * (din + 1), :])
                        for qc in range(NQC):
                            nc.tensor.matmul(
                                pts[qc][:],
                                lhsT=w_sb[:, 128 * din : 128 * (din + 1)],
                                rhs=xt[:, 512 * qc : 512 * (qc + 1)],
                                start=(din == 0),
                                stop=(din == NDIN - 1),
                            )
                    for qc in range(NQC):
                        nc.vector.tensor_scalar_add(
                            dst[:, 512 * qc : 512 * (qc + 1)], pts[qc][:], b_sb[:]
                        )

            # =================== phase 2: attention ===================
            a2a_in = dram.tile([1024, 512], BF16)
            a2a_out = dram.tile([1024, 512], BF16)
            with (
                tc.tile_pool(name="stp", bufs=3, space="PSUM") as stp,  # 6 banks
                tc.tile_pool(name="otp", bufs=1, space="PSUM") as otp,  # 1 bank
                tc.tile_pool(name="rsp", bufs=1, space="PSUM") as rsp,  # 1 bank
                tc.tile_pool(name="ptp", bufs=8) as ptp,
                tc.tile_pool(name="nrm", bufs=2) as nrm,
            ):
                for qc in range(NQC):
                    b = qc // 4
                    q0 = 512 * qc
                    ot = otp.tile([128, 512], F32, name=f"ot{qc}", tag="ot")
                    rs = rsp.tile([128, 512], F32, name=f"rs{qc}", tag="rs")
                    for kt in range(NKT):
                        kk = 2048 * b + 128 * kt
                        vk = 128 * (16 * b + kt)
                        first = kt == 0
                        last = kt == NKT - 1
                        st = stp.tile([128, 1024], F32, name=f"st{qc}_{kt}", tag="st")
                        pt = ptp.tile([128, 1024], BF16, name=f"pt{qc}_{kt}", tag="pt")
                        for h in range(2):
                            nc.tensor.matmul(
                                st[:, 512 * h : 512 * (h + 1)],
                                lhsT=kt_sb[64 * h : 64 * (h + 1), kk : kk + 128],
                                rhs=qt_sb[64 * h : 64 * (h + 1), q0 : q0 + 512],
                                start=True,
                                stop=True,
                            )
                        nc.scalar.activation(pt[:], st[:], AF.Exp, scale=0.125)
                        for h in range(2):
                            nc.tensor.matmul(
                                ot[64 * h : 64 * (h + 1), :],
                                lhsT=v_sb[:, vk + 64 * h : vk + 64 * (h + 1)],
                                rhs=pt[:, 512 * h : 512 * (h + 1)],
                                start=first,
                                stop=last,
                                tile_position=(0, 64 * h),
                            )
                        for h in range(2):
                            nc.tensor.matmul(
                                rs[32 * h : 32 * h + 1, :],
                                lhsT=ones_col[:],
                                rhs=pt[:, 512 * h : 512 * (h + 1)],
                                start=first,
                                stop=last,
                                tile_position=(0, 32 * h),
                            )
                    # drain ot to SBUF immediately so the next q-chunk's PV can
                    # reuse the bank without waiting on the normalize chain
                    ot_sb = nrm.tile([128, 512], F32, name=f"otsb{qc}", tag="otsb")
                    nc.vector.tensor_copy(ot_sb[:], ot[:])
                    # softmax denominators -> broadcast -> normalize + residual
                    rs_bf = nrm.tile([128, 512], BF16, name=f"rsbf{qc}", tag="rsbf")
                    nc.vector.tensor_copy(rs_bf[0:1, :], rs[0:1, :])
                    nc.vector.tensor_copy(rs_bf[32:33, :], rs[32:33, :])
                    bc = rsp.tile([128, 512], F32, name=f"bc{qc}", tag="rs")
                    nc.tensor.matmul(
                        bc[0:64, :], lhsT=ones_sb[0:1, 0:64], rhs=rs_bf[0:1, :],
                        start=True, stop=True, tile_position=(0, 0),
                    )
                    nc.tensor.matmul(
                        bc[64:128, :], lhsT=ones_sb[32:33, 0:64], rhs=rs_bf[32:33, :],
                        start=True, stop=True, tile_position=(32, 64),
                    )
                    recipb = nrm.tile([128, 512], F32, name=f"recipb{qc}", tag="recipb")
                    nc.vector.reciprocal(recipb[:], bc[:])
                    o_tmp = nrm.tile([128, 512], BF16, name=f"otmp{qc}", tag="otmp")
                    nc.vector.tensor_tensor(o_tmp[:], ot_sb[:], recipb[:], ALU.mult)
                    nc.vector.tensor_tensor(
                        zt_local[:, q0 : q0 + 512], o_tmp[:], xres_sb[:, q0 : q0 + 512],
                        ALU.add,
                    )
                    # ship this q-chunk's block to the A2A bounce right away
                    nc.gpsimd.dma_start(
                        a2a_in[128 * qc : 128 * (qc + 1), :],
                        zt_local[:, q0 : q0 + 512],
                    )

            # =================== phase 3: A2A + fc ===================
            nc.gpsimd.collective_compute(
                "AllToAll",
                ALU.bypass,
                replica_groups=[list(range(N_CORES))],
                ins=[a2a_in.opt()],
                outs=[a2a_out.opt()],
            )
            for j in range(NQC):
                nc.gpsimd.dma_start(
                    zt_full[:, 512 * j : 512 * (j + 1)], a2a_out[128 * j : 128 * (j + 1), :]
                )

            with (
                tc.tile_pool(name="fcps", bufs=4, space="PSUM") as fcps,
                tc.tile_pool(name="ysb", bufs=2) as ysb,
            ):
                for qt in range(4):
                    y = ysb.tile([128, 1024], F32, name=f"y{qt}", tag="y")
                    for nb in range(2):
                        yp = fcps.tile([128, 512], F32, name=f"yp{qt}_{nb}", tag="yp")
                        for j in range(NDIN):
                            nc.tensor.matmul(
                                yp[:],
                                lhsT=zt_full[:, 512 * j + 128 * qt : 512 * j + 128 * (qt + 1)],
                                rhs=wf_sb[:, 1024 * j + 512 * nb : 1024 * j + 512 * (nb + 1)],
                                start=(j == 0),
                                stop=(j == NDIN - 1),
                            )
                        nc.vector.tensor_copy(y[:, 512 * nb : 512 * (nb + 1)], yp[:])
                    nc.gpsimd.dma_start(out[128 * qt : 128 * (qt + 1), :], y[:])

    nc.compile()
    return nc


def _numpy_reference(q_, k_, v_, mask, Wq, bq, Wk, bk, Wv, bv, Wf, bf):
    q_ = np.asarray(q_, np.float32)
    k_ = np.asarray(k_, np.float32)
    v_ = np.asarray(v_, np.float32)
    b = q_.shape[0]

    def split(x):
        return x.reshape(b, -1, H, DH).transpose(0, 2, 1, 3)

    q = split(q_ @ Wq + bq)
    k = split(k_ @ Wk + bk)
    v = split(v_ @ Wv + bv)
    attn = np.einsum("bhqd,bhkd->bhqk", q, k) / np.sqrt(np.float32(DH))
    attn = np.where(np.asarray(mask)[:, None, :, None], attn, np.float32(-1e12))
    attn = attn - attn.max(axis=-1, keepdims=True)
    e = np.exp(attn)
    p = e / e.sum(axis=-1, keepdims=True)
    o = np.einsum("bhqk,bhkd->bhqd", p, v)
    o = o.transpose(0, 2, 1, 3).reshape(b, -1, D)
    return (o + q_) @ Wf + bf


def kernel(q_, k_, v_, mask, Wq, bq, Wk, bk, Wv, bv, Wf, bf):
    mask = np.asarray(mask)
    if not mask.all():
        return _numpy_reference(q_, k_, v_, mask, Wq, bq, Wk, bk, Wv, bv, Wf, bf)

    q_ = np.asarray(q_, np.float32)
    k_ = np.asarray(k_, np.float32)
    v_ = np.asarray(v_, np.float32)

    # transposed, b-major-concatenated inputs (shared across cores)
    xq = np.ascontiguousarray(np.concatenate([q_[b].T for b in range(B)], axis=1)).astype(BF16NP)
    xk = np.ascontiguousarray(np.concatenate([k_[b].T for b in range(B)], axis=1)).astype(BF16NP)
    xv = np.ascontiguousarray(np.concatenate([v_[b].T for b in range(B)], axis=1)).astype(BF16NP)
    wf_b = np.ascontiguousarray(np.asarray(Wf, np.float32)).astype(BF16NP)

    in_maps = []
    for c in range(N_CORES):
        d0 = 128 * c
        in_maps.append(
            {
                "xq": xq,
                "xk": xk,
                "xv": xv,
                "xres": np.ascontiguousarray(xq[d0 : d0 + 128, :]),
                "wq": np.ascontiguousarray(np.asarray(Wq, np.float32)[:, d0 : d0 + 128]).astype(BF16NP),
                "wk": np.ascontiguousarray(np.asarray(Wk, np.float32)[:, d0 : d0 + 128]).astype(BF16NP),
                "wv": np.ascontiguousarray(np.asarray(Wv, np.float32)[:, d0 : d0 + 128]).astype(BF16NP),
                "wf": wf_b,
                "bq": np.ascontiguousarray(np.asarray(bq, np.float32)[d0 : d0 + 128, None]),
                "bk": np.ascontiguousarray(np.asarray(bk, np.float32)[d0 : d0 + 128, None]),
                "bv": np.ascontiguousarray(np.asarray(bv, np.float32)[None, d0 : d0 + 128]).astype(BF16NP),
            }
        )

    if "nc" not in _CACHE:
        _CACHE["nc"] = _build()
    res = run_bass_kernel_spmd(_CACHE["nc"], in_maps, core_ids=list(range(N_CORES)))

    out = np.empty((B, S, D), np.float32)
    for c in range(N_CORES):
        y = res.results[c]["out"]
        out[c // 4, 512 * (c % 4) : 512 * (c % 4 + 1), :] = y
    out += np.asarray(bf, np.float32)[None, None, :]
    return out


if __name__ == "__main__":
    # smoke test with small random data through the numpy fallback shapes
    rng = np.random.default_rng(0)
    args = dict(
        q_=rng.standard_normal((B, S, D), dtype=np.float32),
        k_=rng.standard_normal((B, S, D), dtype=np.float32),
        v_=rng.standard_normal((B, S, D), dtype=np.float32),
        mask=np.ones((B, S), bool),
        Wq=rng.standard_normal((D, D), dtype=np.float32) * 0.02,
        bq=np.zeros(D, np.float32),
        Wk=rng.standard_normal((D, D), dtype=np.float32) * 0.02,
        bk=np.zeros(D, np.float32),
        Wv=rng.standard_normal((D, D), dtype=np.float32) * 0.02,
        bv=np.zeros(D, np.float32),
        Wf=rng.standard_normal((D, D), dtype=np.float32) * 0.02,
        bf=np.zeros(D, np.float32),
    )
    got = kernel(**args)
    want = _numpy_reference(**args)
    rel = np.abs(got - want).max() / np.abs(want).max()
    print("rel_err:", rel)
